# revision 7
# baseline (speedup 1.0000x reference)
"""Trainium2 Bass kernel for nn_DenseStationaryQMatrixDecoder.

Reference math: Q = rownorm(exp(logQ) * (1-I)) - I  (a 4x4 CTMC rate matrix),
output = broadcast(row0(expm(Q*1000)), (V, S, A)).  expm(Q*1000) converges to
the rank-1 stationary matrix 1*pi^T, so every output element is pi[a].

Device strategy (per core, 8 cores data-parallel over V):
  1. Build P = rownorm(exp(logQ) with diag zeroed) on-chip.  The host packs
     logQ with -100 added on the diagonal (exp() zeroes the diagonal for
     free).  exp and the row-sum are fused in one scalar-engine activation
     (accum_out); P = E * (1/rowsum) on DVE.
  2. pi = row0(P^8) by repeated squaring.  |lambda2(P)| for sigma=0.1
     logits is ~0.37 (measured 0.374 on the actual seeded input), so P^8
     leaves a relative error of ~4e-4 -- 50x below the 2e-2 gate.
     Squaring without transposes: keep (X, X^T); X2 = matmul(lhsT=X^T,
     rhs=X), X2^T = matmul(lhsT=X, rhs=X^T).  P^T is produced by the DVE
     32x32 stream transpose (block transpose is element-wise, so the
     4x4 in the top-left corner transposes in place; the rest of the
     block is memset garbage nobody reads) -- no PE round-trip.
  3. The final squaring is fused with the partition broadcast:
     row0(X@X) = (XT[:,0])^T @ X, so matmul(lhsT=XT[:,0] bcast to (4,128),
     rhs=X) yields a (128, 4) PSUM tile whose every row is pi.
  4. Tile pi along the free dim into a [128, 2048] SBUF pattern tile
     (8 KiB per partition == the DMA packet size, so output descriptors
     run at full rate).  DVE fills it straight from PSUM in two stages so
     the first output chunk's DMA launches after stage one.  (Two engines
     cannot fill in parallel: the tile framework serializes writers of
     one tile, measured on HW.)
  5. The 8 MiB output shard is written as 8 x 1 MiB chunks alternating
     between the two hardware DGE queues (Sync and Scalar).  Both queues
     feed the SAME 16 DMA engines (measured: Q1/Q10 interleave on
     E64-79 at ~26.7 GB/s each, ~427 GB/s aggregate -- the per-core
     ceiling), but two sequencers generate descriptors in parallel.
     Chunk 0 reads only the stage-one half of the pattern (stride-0
     double read) so its DMA starts ~0.7 us before the fill completes.
"""

import sys

if "/opt/trn_rl_repo" not in sys.path:
    sys.path.insert(0, "/opt/trn_rl_repo")

import numpy as np

A = 4
V = 512
S = 8192
N_CORES = 8
PER_CORE = V * S * A // N_CORES  # 2,097,152 f32 = 8 MiB
P128 = 128
PATT = 2048                      # pattern tile free size (f32) = 8 KiB rows
FREE = 2048                      # output chunk free size (f32)
CHUNKS = PER_CORE // (P128 * FREE)  # 8 chunks of 1 MiB
W = 16                           # pi copies in the final matmul's PSUM tile
NSQ = 3                          # total squarings incl. the fused final one
TR = 32                          # DVE stream-transpose block size

_cache = {}


def _build():
    import concourse.bacc as bacc
    import concourse.mybir as mybir
    import concourse.tile as tile

    f32 = mybir.dt.float32
    AF = mybir.ActivationFunctionType

    nc = bacc.Bacc(
        "TRN2", target_bir_lowering=False, debug=False, num_devices=N_CORES
    )
    blob = nc.dram_tensor("blob", [A, A], f32, kind="ExternalInput").ap()
    out = nc.dram_tensor(
        "out", [CHUNKS, P128, FREE], f32, kind="ExternalOutput"
    ).ap()

    with tile.TileContext(nc) as tc:
        with (
            tc.tile_pool(name="small", bufs=1) as sp,
            tc.tile_pool(name="loop", bufs=2) as lp,
            tc.tile_pool(name="patt", bufs=1) as pp,
            tc.tile_pool(name="ps1", bufs=1, space="PSUM") as ps1,
            tc.tile_pool(name="ps2", bufs=2, space="PSUM") as ps2,
        ):
            lq = sp.tile([A, A], f32)       # logq, diagonal pre-masked to -100
            nc.sync.dma_start(out=lq[:], in_=blob, single_packet=True)

            # 32x32 scratch for the DVE stream transpose; memset off the
            # critical path (runs during the input DMA wait).
            X32 = sp.tile([TR, TR], f32)
            XT32 = sp.tile([TR, TR], f32)
            nc.gpsimd.memset(X32[:], 0.0)

            E = sp.tile([A, A], f32)        # exp(lq): zero diagonal
            s = sp.tile([A, 1], f32)        # fused row sums
            nc.scalar.activation(out=E[:], in_=lq[:], func=AF.Exp, accum_out=s[:])
            r = sp.tile([A, 1], f32)
            nc.vector.reciprocal(out=r[:], in_=s[:])

            # X = P = diag(r) @ E, written into the transpose scratch corner
            X0 = X32[0:A, 0:A]
            nc.vector.tensor_scalar_mul(out=X0, in0=E[:], scalar1=r[:])
            # X^T via DVE 32x32 block transpose (no PE round-trip)
            nc.vector.transpose(out=XT32[:], in_=X32[:])
            XT0 = XT32[0:A, 0:A]

            # Squaring loop.  Both matmuls of an iteration write bank-aligned
            # quads of ONE two-bank PSUM tile, so a single strided DVE copy
            # pulls X2 and X2^T back to SBUF side by side.
            BANK = 512  # f32 elems per PSUM bank row
            Xa, XTa = X0, XT0
            for _ in range(NSQ - 1):
                pr = ps2.tile([A, 2 * BANK], f32)
                nc.tensor.matmul(
                    pr[:, 0:A], lhsT=XTa, rhs=Xa, start=True, stop=True
                )
                nc.tensor.matmul(
                    pr[:, BANK : BANK + A], lhsT=Xa, rhs=XTa,
                    start=True, stop=True,
                )
                pair = lp.tile([A, 2 * A], f32)
                psrc = pr[:].rearrange("p (b f) -> p b f", b=2)[:, :, 0:A]
                pdst = pair[:].rearrange("p (b f) -> p b f", b=2)
                nc.vector.tensor_copy(out=pdst, in_=psrc)
                Xa, XTa = pair[:, 0:A], pair[:, A : 2 * A]

            # Fused last squaring + broadcast:
            # row0(X@X) = (XT[:,0])^T @ X, replicated to 128 partitions by
            # free-dim-broadcasting the stationary operand.  The rhs is X
            # tiled W/A times (a tiny DVE copy that overlaps the matmul's
            # LDWEIGHTS) so PSUM ends up with pi replicated W/A times --
            # the pattern fill then reads W-element runs instead of 4-element
            # runs, which cuts its per-element AP overhead by ~35%.
            xw = sp.tile([A, W], f32)
            nc.vector.tensor_copy(
                out=xw[:].rearrange("p (r a) -> p r a", a=A),
                in_=Xa.unsqueeze(1).to_broadcast((A, W // A, A)),
            )
            pbig = ps1.tile([P128, W], f32)
            nc.tensor.matmul(
                pbig[:],
                lhsT=XTa[:, 0:1].to_broadcast((A, P128)),
                rhs=xw[:],
                start=True,
                stop=True,
            )

            # DVE fills the pattern tile straight from PSUM in one stage;
            # every chunk is a single stride-1 read of it (8 KiB
            # descriptors == the DMA packet size, full engine rate).
            patt = pp.tile([P128, PATT], f32)
            view = patt[:].rearrange("p (r w) -> p r w", w=W)
            nc.vector.tensor_copy(
                out=view,
                in_=pbig[:].unsqueeze(1).to_broadcast((P128, PATT // W, W)),
            )
            for i in range(CHUNKS):
                eng = nc.scalar if (i % 2) else nc.sync
                eng.dma_start(out=out[i], in_=patt[:])

    nc.compile()
    return nc


def _get_nc():
    if "nc" not in _cache:
        _cache["nc"] = _build()
    return _cache["nc"]


def _in_map(log_Q_matrix_AxA):
    logq = np.asarray(log_Q_matrix_AxA, dtype=np.float32).reshape(A, A)
    eye = np.eye(A, dtype=np.float32)
    blob = np.ascontiguousarray(logq - 100.0 * eye)
    return {"blob": blob}


def kernel(
    embeddings_VxD=None, site_positions_SxC=None, log_Q_matrix_AxA=None, **_unused
):
    from concourse.bass_utils import run_bass_kernel_spmd

    nc = _get_nc()
    im = _in_map(log_Q_matrix_AxA)
    res = run_bass_kernel_spmd(
        nc, [dict(im) for _ in range(N_CORES)], core_ids=list(range(N_CORES))
    )
    parts = [r["out"].reshape(V // N_CORES, S, A) for r in res.results]
    return np.concatenate(parts, axis=0)


# revision 11
# speedup vs baseline: 1.0126x; 1.0126x over previous
"""Trainium2 Bass kernel for nn_DenseStationaryQMatrixDecoder.

Reference math: Q = rownorm(exp(logQ) * (1-I)) - I  (a 4x4 CTMC rate matrix),
output = broadcast(row0(expm(Q*1000)), (V, S, A)).  expm(Q*1000) converges to
the rank-1 stationary matrix 1*pi^T, so every output element is pi[a].

Device strategy (per core, 8 cores data-parallel over V):
  1. Build P = rownorm(exp(logQ) with diag zeroed) on-chip.  The host packs
     logQ with -100 added on the diagonal (exp() zeroes the diagonal for
     free).  exp and the row-sum are fused in one scalar-engine activation
     (accum_out); P = E * (1/rowsum) on DVE.
  2. pi = row0(P^8) by repeated squaring.  |lambda2(P)| for sigma=0.1
     logits is ~0.37 (measured 0.374 on the actual seeded input), so P^8
     leaves a relative error of ~4e-4 -- 50x below the 2e-2 gate.
     Squaring without transposes: keep (X, X^T); X2 = matmul(lhsT=X^T,
     rhs=X), X2^T = matmul(lhsT=X, rhs=X^T).  P^T is produced by the DVE
     32x32 stream transpose (block transpose is element-wise, so the
     4x4 in the top-left corner transposes in place; the rest of the
     block is memset garbage nobody reads) -- no PE round-trip.
  3. The final squaring is fused with the partition broadcast:
     row0(X@X) = (XT[:,0])^T @ X, so matmul(lhsT=XT[:,0] bcast to (4,128),
     rhs=X) yields a (128, 4) PSUM tile whose every row is pi.
  4. Tile pi along the free dim into a [128, 1024] SBUF pattern tile.
     DVE fills it straight from PSUM in one stage (two engines cannot
     fill in parallel: the tile framework serializes writers of one
     tile, measured on HW).  4 KiB descriptors run at ~the same
     per-engine rate as 8 KiB ones, and the uniform all-4KiB dual-queue
     layout is the only dual-queue one measured free of a deterministic
     single-engine straggle that 8 KiB dual-queue layouts exhibit.
  5. The 8 MiB output shard is written as 8 x 1 MiB chunks, each a
     stride-0 double read of the pattern tile, alternating between the
     two hardware DGE queues (Sync and Scalar).  Both queues feed the
     SAME 16 DMA engines (measured: Q1/Q10 interleave on E64-79 at
     ~26.7 GB/s each, ~427 GB/s aggregate -- the per-core ceiling), but
     two sequencers generate descriptors in parallel, and the engines
     stay saturated from the first packet to the last.
"""

import sys

if "/opt/trn_rl_repo" not in sys.path:
    sys.path.insert(0, "/opt/trn_rl_repo")

import numpy as np

A = 4
V = 512
S = 8192
N_CORES = 8
PER_CORE = V * S * A // N_CORES  # 2,097,152 f32 = 8 MiB
P128 = 128
PATT = 1024                      # pattern tile free size (f32) = 4 KiB rows
FREE = 2048                      # output chunk free size (f32)
CHUNKS = PER_CORE // (P128 * FREE)  # 8 chunks of 1 MiB
NSQ = 3                          # total squarings incl. the fused final one
TR = 32                          # DVE stream-transpose block size

_cache = {}


def _build():
    import concourse.bacc as bacc
    import concourse.mybir as mybir
    import concourse.tile as tile

    f32 = mybir.dt.float32
    AF = mybir.ActivationFunctionType

    nc = bacc.Bacc(
        "TRN2", target_bir_lowering=False, debug=False, num_devices=N_CORES
    )
    blob = nc.dram_tensor("blob", [A, A], f32, kind="ExternalInput").ap()
    out = nc.dram_tensor(
        "out", [CHUNKS, P128, FREE], f32, kind="ExternalOutput"
    ).ap()

    with tile.TileContext(nc) as tc:
        with (
            tc.tile_pool(name="small", bufs=1) as sp,
            tc.tile_pool(name="loop", bufs=2) as lp,
            tc.tile_pool(name="patt", bufs=1) as pp,
            tc.tile_pool(name="ps1", bufs=1, space="PSUM") as ps1,
            tc.tile_pool(name="ps2", bufs=2, space="PSUM") as ps2,
        ):
            lq = sp.tile([A, A], f32)       # logq, diagonal pre-masked to -100
            nc.sync.dma_start(out=lq[:], in_=blob, single_packet=True)

            # 32x32 scratch for the DVE stream transpose; memset off the
            # critical path (runs during the input DMA wait).
            X32 = sp.tile([TR, TR], f32)
            XT32 = sp.tile([TR, TR], f32)
            nc.gpsimd.memset(X32[:], 0.0)

            E = sp.tile([A, A], f32)        # exp(lq): zero diagonal
            s = sp.tile([A, 1], f32)        # fused row sums
            nc.scalar.activation(out=E[:], in_=lq[:], func=AF.Exp, accum_out=s[:])
            r = sp.tile([A, 1], f32)
            nc.vector.reciprocal(out=r[:], in_=s[:])

            # X = P = diag(r) @ E, written into the transpose scratch corner
            X0 = X32[0:A, 0:A]
            nc.vector.tensor_scalar_mul(out=X0, in0=E[:], scalar1=r[:])
            # X^T via DVE 32x32 block transpose (no PE round-trip)
            nc.vector.transpose(out=XT32[:], in_=X32[:])
            XT0 = XT32[0:A, 0:A]

            # Squaring loop.  Both matmuls of an iteration write bank-aligned
            # quads of ONE two-bank PSUM tile, so a single strided DVE copy
            # pulls X2 and X2^T back to SBUF side by side.
            BANK = 512  # f32 elems per PSUM bank row
            Xa, XTa = X0, XT0
            for _ in range(NSQ - 1):
                pr = ps2.tile([A, 2 * BANK], f32)
                nc.tensor.matmul(
                    pr[:, 0:A], lhsT=XTa, rhs=Xa, start=True, stop=True
                )
                nc.tensor.matmul(
                    pr[:, BANK : BANK + A], lhsT=Xa, rhs=XTa,
                    start=True, stop=True,
                )
                pair = lp.tile([A, 2 * A], f32)
                psrc = pr[:].rearrange("p (b f) -> p b f", b=2)[:, :, 0:A]
                pdst = pair[:].rearrange("p (b f) -> p b f", b=2)
                nc.vector.tensor_copy(out=pdst, in_=psrc)
                Xa, XTa = pair[:, 0:A], pair[:, A : 2 * A]

            # Fused last squaring + broadcast:
            # row0(X@X) = (XT[:,0])^T @ X, replicated to 128 partitions by
            # free-dim-broadcasting the stationary operand.
            pbig = ps1.tile([P128, A], f32)
            nc.tensor.matmul(
                pbig[:],
                lhsT=XTa[:, 0:1].to_broadcast((A, P128)),
                rhs=Xa,
                start=True,
                stop=True,
            )

            # DVE fills the pattern tile straight from PSUM in one stage;
            # every chunk reads it twice (stride-0).  4 KiB descriptors
            # run at ~same per-engine rate as 8 KiB ones, and the uniform
            # all-4KiB dual-queue layout is the only one measured free of
            # the deterministic single-engine (E79) straggle that 8 KiB
            # dual-queue layouts exhibit.
            patt = pp.tile([P128, PATT], f32)
            view = patt[:].rearrange("p (r a) -> p r a", a=A)
            nc.vector.tensor_copy(
                out=view,
                in_=pbig[:].unsqueeze(1).to_broadcast((P128, PATT // A, A)),
            )
            src = patt[:].unsqueeze(1).to_broadcast((P128, 2, PATT))
            for i in range(CHUNKS):
                eng = nc.scalar if (i % 2) else nc.sync
                eng.dma_start(
                    out=out[i].rearrange("p (c f) -> p c f", c=2), in_=src
                )

    nc.compile()
    return nc


def _get_nc():
    if "nc" not in _cache:
        _cache["nc"] = _build()
    return _cache["nc"]


def _in_map(log_Q_matrix_AxA):
    logq = np.asarray(log_Q_matrix_AxA, dtype=np.float32).reshape(A, A)
    eye = np.eye(A, dtype=np.float32)
    blob = np.ascontiguousarray(logq - 100.0 * eye)
    return {"blob": blob}


def kernel(
    embeddings_VxD=None, site_positions_SxC=None, log_Q_matrix_AxA=None, **_unused
):
    from concourse.bass_utils import run_bass_kernel_spmd

    nc = _get_nc()
    im = _in_map(log_Q_matrix_AxA)
    res = run_bass_kernel_spmd(
        nc, [dict(im) for _ in range(N_CORES)], core_ids=list(range(N_CORES))
    )
    parts = [r["out"].reshape(V // N_CORES, S, A) for r in res.results]
    return np.concatenate(parts, axis=0)


# revision 13
# speedup vs baseline: 1.0352x; 1.0224x over previous
"""Trainium2 Bass kernel for nn_DenseStationaryQMatrixDecoder.

Reference math: Q = rownorm(exp(logQ) * (1-I)) - I  (a 4x4 CTMC rate matrix),
output = broadcast(row0(expm(Q*1000)), (V, S, A)).  expm(Q*1000) converges to
the rank-1 stationary matrix 1*pi^T, so every output element is pi[a].

Device strategy (per core, 8 cores data-parallel over V):
  1. Build P = rownorm(exp(logQ) with diag zeroed) on-chip.  The host packs
     logQ with -100 added on the diagonal (exp() zeroes the diagonal for
     free).  exp and the row-sum are fused in one scalar-engine activation
     (accum_out); P = E * (1/rowsum) on DVE.
  2. pi = row0(P^8) by repeated squaring.  |lambda2(P)| for sigma=0.1
     logits is ~0.37 (measured 0.374 on the actual seeded input), so P^8
     leaves a relative error of ~4e-4 -- 50x below the 2e-2 gate.
     Squaring without transposes: keep (X, X^T); X2 = matmul(lhsT=X^T,
     rhs=X), X2^T = matmul(lhsT=X, rhs=X^T).  P^T is produced by the DVE
     32x32 stream transpose (block transpose is element-wise, so the
     4x4 in the top-left corner transposes in place; the rest of the
     block is memset garbage nobody reads) -- no PE round-trip.
  3. The final squaring is fused with the partition broadcast:
     row0(X@X) = (XT[:,0])^T @ X, so matmul(lhsT=XT[:,0] bcast to (4,128),
     rhs=X) yields a (128, 4) PSUM tile whose every row is pi.
  4. Tile pi along the free dim into a [128, 1024] SBUF pattern tile.
     DVE fills it straight from PSUM in one stage (two engines cannot
     fill in parallel: the tile framework serializes writers of one
     tile, measured on HW).  4 KiB descriptors run at ~the same
     per-engine rate as 8 KiB ones, and the uniform all-4KiB dual-queue
     layout is the only dual-queue one measured free of a deterministic
     single-engine straggle that 8 KiB dual-queue layouts exhibit.
  5. The 8 MiB output shard is written as 8 x 1 MiB chunks, each a
     stride-0 double read of the pattern tile, alternating between the
     two hardware DGE queues (Sync and Scalar).  Both queues feed the
     SAME 16 DMA engines (measured: Q1/Q10 interleave on E64-79 at
     ~26.7 GB/s each, ~427 GB/s aggregate -- the per-core ceiling), but
     two sequencers generate descriptors in parallel, and the engines
     stay saturated from the first packet to the last.
"""

import sys

if "/opt/trn_rl_repo" not in sys.path:
    sys.path.insert(0, "/opt/trn_rl_repo")

import numpy as np

A = 4
V = 512
S = 8192
N_CORES = 8
PER_CORE = V * S * A // N_CORES  # 2,097,152 f32 = 8 MiB
P128 = 128
PATT = 2048                      # pattern tile free size (f32) = 8 KiB rows
FREE = 2048                      # output chunk free size (f32)
CHUNKS = PER_CORE // (P128 * FREE)  # 8 chunks of 1 MiB
H = 1024                         # fill stage size (f32)
NSQ = 3                          # total squarings incl. the fused final one
TR = 32                          # DVE stream-transpose block size

_cache = {}


def _build():
    import concourse.bacc as bacc
    import concourse.mybir as mybir
    import concourse.tile as tile

    f32 = mybir.dt.float32
    AF = mybir.ActivationFunctionType

    nc = bacc.Bacc(
        "TRN2", target_bir_lowering=False, debug=False, num_devices=N_CORES
    )
    blob = nc.dram_tensor("blob", [A, A], f32, kind="ExternalInput").ap()
    out = nc.dram_tensor(
        "out", [CHUNKS, P128, FREE], f32, kind="ExternalOutput"
    ).ap()

    with tile.TileContext(nc) as tc:
        with (
            tc.tile_pool(name="small", bufs=1) as sp,
            tc.tile_pool(name="loop", bufs=2) as lp,
            tc.tile_pool(name="patt", bufs=1) as pp,
            tc.tile_pool(name="ps1", bufs=1, space="PSUM") as ps1,
            tc.tile_pool(name="ps2", bufs=2, space="PSUM") as ps2,
        ):
            lq = sp.tile([A, A], f32)       # logq, diagonal pre-masked to -100
            nc.sync.dma_start(out=lq[:], in_=blob, single_packet=True)

            # 32x32 scratch for the DVE stream transpose; memset off the
            # critical path (runs during the input DMA wait).
            X32 = sp.tile([TR, TR], f32)
            XT32 = sp.tile([TR, TR], f32)
            nc.gpsimd.memset(X32[:], 0.0)

            E = sp.tile([A, A], f32)        # exp(lq): zero diagonal
            s = sp.tile([A, 1], f32)        # fused row sums
            nc.scalar.activation(out=E[:], in_=lq[:], func=AF.Exp, accum_out=s[:])
            r = sp.tile([A, 1], f32)
            nc.vector.reciprocal(out=r[:], in_=s[:])

            # X = P = diag(r) @ E, written into the transpose scratch corner
            X0 = X32[0:A, 0:A]
            nc.vector.tensor_scalar_mul(out=X0, in0=E[:], scalar1=r[:])
            # X^T via DVE 32x32 block transpose (no PE round-trip)
            nc.vector.transpose(out=XT32[:], in_=X32[:])
            XT0 = XT32[0:A, 0:A]

            # Squaring loop.  Both matmuls of an iteration write bank-aligned
            # quads of ONE two-bank PSUM tile, so a single strided DVE copy
            # pulls X2 and X2^T back to SBUF side by side.
            BANK = 512  # f32 elems per PSUM bank row
            Xa, XTa = X0, XT0
            for _ in range(NSQ - 1):
                pr = ps2.tile([A, 2 * BANK], f32)
                nc.tensor.matmul(
                    pr[:, 0:A], lhsT=XTa, rhs=Xa, start=True, stop=True
                )
                nc.tensor.matmul(
                    pr[:, BANK : BANK + A], lhsT=Xa, rhs=XTa,
                    start=True, stop=True,
                )
                pair = lp.tile([A, 2 * A], f32)
                psrc = pr[:].rearrange("p (b f) -> p b f", b=2)[:, :, 0:A]
                pdst = pair[:].rearrange("p (b f) -> p b f", b=2)
                nc.vector.tensor_copy(out=pdst, in_=psrc)
                Xa, XTa = pair[:, 0:A], pair[:, A : 2 * A]

            # Fused last squaring + broadcast:
            # row0(X@X) = (XT[:,0])^T @ X, replicated to 128 partitions by
            # free-dim-broadcasting the stationary operand.
            pbig = ps1.tile([P128, A], f32)
            nc.tensor.matmul(
                pbig[:],
                lhsT=XTa[:, 0:1].to_broadcast((A, P128)),
                rhs=Xa,
                start=True,
                stop=True,
            )

            # DVE fills the pattern tile straight from PSUM in two stages.
            # Chunk 0 goes out as a stride-0 double read of the stage-one
            # half (4 KiB descriptors) as soon as that half is ready, so
            # its DMA overlaps the second fill stage; chunks 1-7 are
            # single stride-1 reads of the full tile (8 KiB descriptors ==
            # the DMA packet size, ~7% better bus efficiency than 4 KiB).
            # Everything rides the Sync HWDGE queue: descriptor generation
            # (~0.7 us/chunk) stays far ahead of the ~2.4 us/chunk drain.
            patt = pp.tile([P128, PATT], f32)
            for fi in range(2):
                view = patt[:, fi * H : (fi + 1) * H].rearrange(
                    "p (r a) -> p r a", a=A
                )
                nc.vector.tensor_copy(
                    out=view,
                    in_=pbig[:].unsqueeze(1).to_broadcast((P128, H // A, A)),
                )
                if fi == 0:
                    src0 = patt[:, 0:H].unsqueeze(1).to_broadcast((P128, 2, H))
                    nc.sync.dma_start(
                        out=out[0].rearrange("p (c f) -> p c f", c=2), in_=src0
                    )
            for i in range(1, CHUNKS):
                nc.sync.dma_start(out=out[i], in_=patt[:])

    nc.compile()
    return nc


def _get_nc():
    if "nc" not in _cache:
        _cache["nc"] = _build()
    return _cache["nc"]


def _in_map(log_Q_matrix_AxA):
    logq = np.asarray(log_Q_matrix_AxA, dtype=np.float32).reshape(A, A)
    eye = np.eye(A, dtype=np.float32)
    blob = np.ascontiguousarray(logq - 100.0 * eye)
    return {"blob": blob}


def kernel(
    embeddings_VxD=None, site_positions_SxC=None, log_Q_matrix_AxA=None, **_unused
):
    from concourse.bass_utils import run_bass_kernel_spmd

    nc = _get_nc()
    im = _in_map(log_Q_matrix_AxA)
    res = run_bass_kernel_spmd(
        nc, [dict(im) for _ in range(N_CORES)], core_ids=list(range(N_CORES))
    )
    parts = [r["out"].reshape(V // N_CORES, S, A) for r in res.results]
    return np.concatenate(parts, axis=0)


# revision 14
# speedup vs baseline: 1.1560x; 1.1167x over previous
"""Trainium2 Bass kernel for nn_DenseStationaryQMatrixDecoder.

Reference math: Q = rownorm(exp(logQ) * (1-I)) - I  (a 4x4 CTMC rate matrix),
output = broadcast(row0(expm(Q*1000)), (V, S, A)).  expm(Q*1000) converges to
the rank-1 stationary matrix 1*pi^T, so every output element is pi[a].

Device strategy (per core, 8 cores data-parallel over V):
  1. Build P = rownorm(exp(logQ) with diag zeroed) on-chip.  The host packs
     logQ with -100 added on the diagonal (exp() zeroes the diagonal for
     free).  exp and the row-sum are fused in one scalar-engine activation
     (accum_out); P = E * (1/rowsum) on DVE.
  2. pi = row0(P^8) by repeated squaring.  |lambda2(P)| for sigma=0.1
     logits is ~0.37 (measured 0.374 on the actual seeded input), so P^8
     leaves a relative error of ~4e-4 -- 50x below the 2e-2 gate.
     Squaring without transposes: keep (X, X^T); X2 = matmul(lhsT=X^T,
     rhs=X), X2^T = matmul(lhsT=X, rhs=X^T).  P^T is produced by the DVE
     32x32 stream transpose (block transpose is element-wise, so the
     4x4 in the top-left corner transposes in place; the rest of the
     block is memset garbage nobody reads) -- no PE round-trip.
  3. The final squaring is fused with the partition broadcast:
     row0(X@X) = (XT[:,0])^T @ X, so matmul(lhsT=XT[:,0] bcast to (4,128),
     rhs=X) yields a (128, 4) PSUM tile whose every row is pi.
  4. Tile pi along the free dim into a [128, 2048] SBUF pattern tile.
     DVE fills it straight from PSUM in two stages (a second engine
     cannot help: the tile framework serializes writers of one tile,
     measured on HW).
  5. The 8 MiB output shard is written as 8 x 1 MiB chunks on the Sync
     HWDGE queue, which fans out over all 16 DMA engines at ~26.7 GB/s
     each (~427 GB/s aggregate -- the per-core ceiling; a second HWDGE
     queue adds no engines, measured).  Chunk 0 is a stride-0 double
     read of the stage-one half and launches while stage two still
     fills; chunks 1-7 are single stride-1 reads of the full tile whose
     8 KiB descriptors match the DMA packet size (~7% better bus
     efficiency than 4 KiB).  Engines stay saturated first packet to
     last, so exec time = first-packet latency + 8 MiB / 427 GB/s +
     the fixed NEFF preamble/semaphore-teardown overhead.
"""

import sys

if "/opt/trn_rl_repo" not in sys.path:
    sys.path.insert(0, "/opt/trn_rl_repo")

import numpy as np

A = 4
V = 512
S = 8192
N_CORES = 8
PER_CORE = V * S * A // N_CORES  # 2,097,152 f32 = 8 MiB
P128 = 128
PATT = 2048                      # pattern tile free size (f32) = 8 KiB rows
FREE = 2048                      # output chunk free size (f32)
CHUNKS = PER_CORE // (P128 * FREE)  # 8 chunks of 1 MiB
H = 1024                         # fill stage size (f32)
NSQ = 3                          # total squarings incl. the fused final one
TR = 32                          # DVE stream-transpose block size

_cache = {}


def _build():
    import concourse.bacc as bacc
    import concourse.mybir as mybir
    import concourse.tile as tile

    f32 = mybir.dt.float32
    AF = mybir.ActivationFunctionType

    nc = bacc.Bacc(
        "TRN2", target_bir_lowering=False, debug=False, num_devices=N_CORES
    )
    blob = nc.dram_tensor("blob", [A, A], f32, kind="ExternalInput").ap()
    out = nc.dram_tensor(
        "out", [CHUNKS, P128, FREE], f32, kind="ExternalOutput"
    ).ap()

    with tile.TileContext(nc) as tc:
        with (
            tc.tile_pool(name="small", bufs=1) as sp,
            tc.tile_pool(name="loop", bufs=2) as lp,
            tc.tile_pool(name="patt", bufs=1) as pp,
            tc.tile_pool(name="ps1", bufs=1, space="PSUM") as ps1,
            tc.tile_pool(name="ps2", bufs=2, space="PSUM") as ps2,
        ):
            lq = sp.tile([A, A], f32)       # logq, diagonal pre-masked to -100
            nc.sync.dma_start(out=lq[:], in_=blob, single_packet=True)

            # 32x32 scratch for the DVE stream transpose; memset off the
            # critical path (runs during the input DMA wait).
            X32 = sp.tile([TR, TR], f32)
            XT32 = sp.tile([TR, TR], f32)
            nc.gpsimd.memset(X32[:], 0.0)

            E = sp.tile([A, A], f32)        # exp(lq): zero diagonal
            s = sp.tile([A, 1], f32)        # fused row sums
            nc.scalar.activation(out=E[:], in_=lq[:], func=AF.Exp, accum_out=s[:])
            r = sp.tile([A, 1], f32)
            nc.vector.reciprocal(out=r[:], in_=s[:])

            # X = P = diag(r) @ E, written into the transpose scratch corner
            X0 = X32[0:A, 0:A]
            nc.vector.tensor_scalar_mul(out=X0, in0=E[:], scalar1=r[:])
            # X^T via DVE 32x32 block transpose (no PE round-trip)
            nc.vector.transpose(out=XT32[:], in_=X32[:])
            XT0 = XT32[0:A, 0:A]

            # Squaring loop.  Both matmuls of an iteration write bank-aligned
            # quads of ONE two-bank PSUM tile, so a single strided DVE copy
            # pulls X2 and X2^T back to SBUF side by side.
            BANK = 512  # f32 elems per PSUM bank row
            Xa, XTa = X0, XT0
            for _ in range(NSQ - 1):
                pr = ps2.tile([A, 2 * BANK], f32)
                nc.tensor.matmul(
                    pr[:, 0:A], lhsT=XTa, rhs=Xa, start=True, stop=True
                )
                nc.tensor.matmul(
                    pr[:, BANK : BANK + A], lhsT=Xa, rhs=XTa,
                    start=True, stop=True,
                )
                pair = lp.tile([A, 2 * A], f32)
                psrc = pr[:].rearrange("p (b f) -> p b f", b=2)[:, :, 0:A]
                pdst = pair[:].rearrange("p (b f) -> p b f", b=2)
                nc.vector.tensor_copy(out=pdst, in_=psrc)
                Xa, XTa = pair[:, 0:A], pair[:, A : 2 * A]

            # Fused last squaring + broadcast:
            # row0(X@X) = (XT[:,0])^T @ X, replicated to 128 partitions by
            # free-dim-broadcasting the stationary operand.
            pbig = ps1.tile([P128, A], f32)
            nc.tensor.matmul(
                pbig[:],
                lhsT=XTa[:, 0:1].to_broadcast((A, P128)),
                rhs=Xa,
                start=True,
                stop=True,
            )

            # DVE fills the pattern tile straight from PSUM in two stages.
            # Chunk 0 goes out as a stride-0 double read of the stage-one
            # half (4 KiB descriptors) as soon as that half is ready, so
            # its DMA overlaps the second fill stage; chunks 1-7 are
            # single stride-1 reads of the full tile (8 KiB descriptors ==
            # the DMA packet size, ~7% better bus efficiency than 4 KiB).
            # Everything rides the Sync HWDGE queue: descriptor generation
            # (~0.7 us/chunk) stays far ahead of the ~2.4 us/chunk drain.
            patt = pp.tile([P128, PATT], f32)
            for fi in range(2):
                view = patt[:, fi * H : (fi + 1) * H].rearrange(
                    "p (r a) -> p r a", a=A
                )
                nc.vector.tensor_copy(
                    out=view,
                    in_=pbig[:].unsqueeze(1).to_broadcast((P128, H // A, A)),
                )
                if fi == 0:
                    src0 = patt[:, 0:H].unsqueeze(1).to_broadcast((P128, 2, H))
                    nc.sync.dma_start(
                        out=out[0].rearrange("p (c f) -> p c f", c=2), in_=src0
                    )
            for i in range(1, CHUNKS):
                nc.sync.dma_start(out=out[i], in_=patt[:])

    nc.compile()
    return nc


def _get_nc():
    if "nc" not in _cache:
        _cache["nc"] = _build()
    return _cache["nc"]


def _in_map(log_Q_matrix_AxA):
    logq = np.asarray(log_Q_matrix_AxA, dtype=np.float32).reshape(A, A)
    eye = np.eye(A, dtype=np.float32)
    blob = np.ascontiguousarray(logq - 100.0 * eye)
    return {"blob": blob}


def kernel(
    embeddings_VxD=None, site_positions_SxC=None, log_Q_matrix_AxA=None, **_unused
):
    from concourse.bass_utils import run_bass_kernel_spmd

    nc = _get_nc()
    im = _in_map(log_Q_matrix_AxA)
    res = run_bass_kernel_spmd(
        nc, [dict(im) for _ in range(N_CORES)], core_ids=list(range(N_CORES))
    )
    parts = [r["out"].reshape(V // N_CORES, S, A) for r in res.results]
    return np.concatenate(parts, axis=0)
